# revision 10
# baseline (speedup 1.0000x reference)
"""Trainium2 Bass kernel for nn_AttnDecoderRNN (sparse_attention).

Data-parallel over 8 NeuronCores: batch 32 -> 4 per core; weights
replicated. BatchNorm batch statistics are globalized with a tiny
(128x4 f32) AllReduce. Heavy matmuls (1024->256 linear over B*16*64
positions, 3x3 256->256 conv as 9 shifted matmuls) run in bf16 with
fp32 PSUM accumulation.

Self-contained: hardcodes all shapes from the problem spec.
"""

import os
import numpy as np
import ml_dtypes

import concourse.bass as bass
import concourse.mybir as mybir
from concourse import bacc
from concourse.tile import TileContext
from concourse.bass_utils import run_bass_kernel_spmd

F32 = mybir.dt.float32
F32R = mybir.dt.float32r
BF16 = mybir.dt.bfloat16
AF = mybir.ActivationFunctionType
ALU = mybir.AluOpType
AX = mybir.AxisListType

N_CORES = 8
B = 4            # per-core batch
HD, WD = 16, 64
HW = HD * WD     # 1024
HSZ = 256
ENC = 1024
VOCAB = 128
P = 128
# padded spatial layout per sample: rows 0..17 (pad/16 interior/pad),
# cols 0..65 (pad/64 interior/pad)
SR, SC = HD + 2, WD + 2
SP = SR * SC                    # 1188
GUARD = SC + 1                  # 67: conv taps read at most +-(66+1)
SPBUF = 2 * GUARD + B * SP      # padded spatial buffer incl guards
NTOT = float(32 * HW)           # BN stat count (global batch)

_CACHED = None
LAST_EXEC_NS = None


def _build():
    nc = bacc.Bacc(None, target_bir_lowering=False, debug=False)

    def din(name, shape, dt=F32):
        return nc.declare_dram_parameter(name, list(shape), dt, isOutput=False)

    def dout(name, shape, dt=F32):
        return nc.declare_dram_parameter(name, list(shape), dt, isOutput=True)

    # ---- inputs (per-core shards / replicated weights) ----
    enc_e = din("enc", [B, ENC, HW], BF16)
    embT_bf_e = din("embT_bf", [P, 2, B], BF16)
    hT_bf_e = din("hT_bf", [P, 2, B], BF16)
    vT_bf_e = din("vT_bf", [P, 6, B], BF16)
    h_row_e = din("h_row", [B, HSZ])
    att_bf_e = din("att_bf", [1, B * HW], BF16)
    mask_e = din("mask", [P, B * SP])

    g1wi_e = din("g1wi", [P, 2, 768], BF16)
    g1wh_e = din("g1wh", [P, 2, 768], BF16)
    g1wv_e = din("g1wv", [P, 6, 768], BF16)
    gwi_e = din("gwi", [P, 8, 768], BF16)
    gwh_e = din("gwh", [P, 2, 768], BF16)
    gwv_e = din("gwv", [P, 6, 768], BF16)
    hidw_e = din("hidw", [P, 2, 256], BF16)
    hidb_e = din("hidb", [P, 2])
    uaw_e = din("uaw", [P, 8, 256], BF16)
    uaufb_e = din("uaufb", [P, 2])
    ufw_e = din("ufw", [1, 256], BF16)
    convw_e = din("convw", [P, 9, 2, 256], BF16)
    convb_e = din("convb", [P, 2])
    bng_e = din("bng", [P, 2])
    bnb_e = din("bnb", [P, 2])
    vattw_e = din("vattw", [P, 2], BF16)
    vattb_e = din("vattb", [1, 1])
    h2w_e = din("h2w", [P, 2, 128], BF16)
    e2w_e = din("e2w", [P, 2, 128], BF16)
    wcw_e = din("wcw", [P, 8, 128], BF16)
    preb_e = din("preb", [P, 1])
    outw_e = din("outw", [P, 128], BF16)
    outb_bf_e = din("outb_bf", [1, 128], BF16)
    ones_e = din("ones", [1, P], BF16)
    ones4_bf_e = din("ones4_bf", [1, B], BF16)
    id4_e = din("id4", [B, B])

    # ---- outputs ----
    o_e = dout("o_out", [B, VOCAB])
    h1_e = dout("h1_out", [B, HSZ])
    v1_e = dout("v1_out", [B, 3 * HSZ])
    al_e = dout("alpha_out", [B, HW])

    with TileContext(nc) as tc:
        with (
            tc.tile_pool(name="w", bufs=1) as wp,
            tc.tile_pool(name="big", bufs=1) as bigp,
            tc.tile_pool(name="encq", bufs=2) as encqp,
            tc.tile_pool(name="enc2", bufs=2) as enc2p,
            tc.tile_pool(name="sq", bufs=4) as sqp,
            tc.tile_pool(name="sm", bufs=1) as smp,
            tc.tile_pool(name="ps", bufs=4, space="PSUM") as psp,
            tc.tile_pool(name="ps2", bufs=1, space="PSUM") as ps2p,
            tc.tile_pool(name="dram", bufs=1, space="DRAM") as dramp,
        ):
            def load(pool, ext, dt=None, shape=None):
                t = pool.tile(shape or list(ext.shape), dt or ext.dtype,
                              tag=f"ld_{ext.name}")
                nc.gpsimd.dma_start(t[:], ext[...])
                return t

            # ---------- small inputs / weights ----------
            embT_bf = load(wp, embT_bf_e)
            hT_bf = load(wp, hT_bf_e)
            vT_bf = load(wp, vT_bf_e)
            h_row = load(wp, h_row_e)
            g1wi = load(wp, g1wi_e)
            g1wh = load(wp, g1wh_e)
            g1wv = load(wp, g1wv_e)
            hidw = load(wp, hidw_e)
            hidb = load(wp, hidb_e)
            uaufb = load(wp, uaufb_e)
            ufw = load(wp, ufw_e)
            att_bf = load(wp, att_bf_e)
            uaw = load(wp, uaw_e)
            convw = load(wp, convw_e)
            convb = load(wp, convb_e)
            id4 = load(wp, id4_e)
            ones = load(wp, ones_e)
            ones4_bf = load(wp, ones4_bf_e)

            mask = bigp.tile([P, B * SP], F32, tag="maskslot")
            nc.gpsimd.dma_start(mask[:], mask_e[...])

            et = bigp.tile([P, 2, SPBUF], BF16)
            nc.gpsimd.memset(et[:], 0.0)

            # ---------- GRU1 (row form, [4, .] tiles) ----------
            def gru_cell(nx, xT_tiles, xw, hT_tiles, hw_, nv, vT_tiles, vw,
                         vg_row, h_in_row, h_out_row):
                # A = x@wi + v@wv (+ h@wh for r,z chunks); Bn = (h@wh)[n]
                A = ps2p.tile([B, 768], F32, tag="gruA")
                Bn = ps2p.tile([B, 256], F32, tag="gruB")
                for ch in range(3):
                    sl = slice(ch * 256, (ch + 1) * 256)
                    mms = [(xT_tiles[:, kt, :], xw[:, kt, sl]) for kt in range(nx)]
                    mms += [(vT_tiles[:, kt, :], vw[:, kt, sl]) for kt in range(nv)]
                    if ch < 2:
                        mms += [(hT_tiles[:, kt, :], hw_[:, kt, sl]) for kt in range(2)]
                    for i, (l, r) in enumerate(mms):
                        nc.tensor.matmul(A[:, sl], l, r,
                                         start=(i == 0), stop=(i == len(mms) - 1))
                for kt in range(2):
                    nc.tensor.matmul(Bn[:, :], hT_tiles[:, kt, :], hw_[:, kt, 512:768],
                                     start=(kt == 0), stop=(kt == 1))
                # gates
                nc.scalar.activation(vg_row[:, 0:256], A[:, 0:256], AF.Sigmoid)
                nc.scalar.activation(vg_row[:, 256:512], A[:, 256:512], AF.Sigmoid)
                tt = smp.tile([B, 256], F32, tag="gr_tt")
                nc.vector.tensor_tensor(out=tt[:], in0=vg_row[:, 0:256], in1=Bn[:, :], op=ALU.mult)
                t2 = smp.tile([B, 256], F32, tag="gr_t2")
                nc.vector.tensor_tensor(out=t2[:], in0=A[:, 512:768], in1=tt[:], op=ALU.add)
                nc.scalar.activation(vg_row[:, 512:768], t2[:], AF.Tanh)
                u = smp.tile([B, 256], F32, tag="gr_u")
                nc.vector.tensor_tensor(out=u[:], in0=h_in_row[:], in1=vg_row[:, 512:768], op=ALU.subtract)
                w2 = smp.tile([B, 256], F32, tag="gr_w2")
                nc.vector.tensor_tensor(out=w2[:], in0=u[:], in1=vg_row[:, 256:512], op=ALU.mult)
                nc.vector.tensor_tensor(out=h_out_row[:], in0=w2[:], in1=vg_row[:, 512:768], op=ALU.add)

            v1g_row = smp.tile([B, 768], F32, tag="v1g")   # gru1 gate concat (= gru v input)
            st_row = smp.tile([B, 256], F32, tag="st")
            gru_cell(2, embT_bf, g1wi, hT_bf, g1wh, 6, vT_bf, g1wv, v1g_row, h_row, st_row)

            def transpose_to_bf(row_ap, ncols, tag):
                # row_ap: [B, ncols*128] f32 -> [128, ncols, B] bf16
                out_bf = smp.tile([P, ncols, B], BF16, tag=tag)
                for t in range(ncols):
                    pt = psp.tile([P, B], F32, tag="mm")
                    nc.tensor.transpose(pt[:], row_ap[:, t * P:(t + 1) * P], id4[:])
                    nc.scalar.activation(out_bf[:, t, :], pt[:], AF.Copy)
                return out_bf

            stT_bf = transpose_to_bf(st_row, 2, "stT")
            vtT_bf = transpose_to_bf(v1g_row, 6, "vtT")

            # hidden1T = (st @ hidden_w + b).T   [128, 2, 4] f32
            bias_comb = smp.tile([P, 2, B], F32, tag="bcomb")
            for mt in range(2):
                ph = psp.tile([P, B], F32, tag="mm")
                for kt in range(2):
                    nc.tensor.matmul(ph[:], hidw[:, kt, mt * P:(mt + 1) * P], stT_bf[:, kt, :],
                                     start=(kt == 0), stop=(kt == 1))
                h1t = smp.tile([P, B], F32, tag="h1t")
                nc.scalar.activation(h1t[:], ph[:], AF.Identity, bias=hidb[:, mt:mt + 1])
                nc.vector.tensor_scalar_add(bias_comb[:, mt, :], h1t[:], uaufb[:, mt:mt + 1])

            # ---------- phase A: ua/uf -> et ----------
            for q in range(4):
                for bp in range(2):
                    encq = encqp.tile([P, 2, 8, 256], BF16, tag="encq")
                    for bi in range(2):
                        b = bp * 2 + bi
                        nc.gpsimd.dma_start(
                            encq[:, bi, :, :],
                            enc_e[b].rearrange("(t p) hw -> p t hw", p=P)[:, :, q * 256:(q + 1) * 256])
                    for bi in range(2):
                        b = bp * 2 + bi
                        for ct in range(2):
                            pg = psp.tile([P, 256], F32, tag="mm")
                            for kt in range(8):
                                nc.tensor.matmul(pg[:], uaw[:, kt, ct * P:(ct + 1) * P],
                                                 encq[:, bi, kt, :], start=(kt == 0), stop=False)
                            nc.tensor.matmul(pg[:], ufw[0:1, ct * P:(ct + 1) * P],
                                             att_bf[0:1, b * HW + q * 256:b * HW + (q + 1) * 256],
                                             start=False, stop=True)
                            base = GUARD + b * SP + (1 + q * 4) * SC + 1
                            out_ap = et[:, ct, base:base + 4 * SC].rearrange(
                                "p (r c) -> p r c", c=SC)[:, :, 0:WD]
                            in_ap = pg[:].rearrange("p (r c) -> p r c", c=WD)
                            nc.scalar.activation(out_ap, in_ap, AF.Identity,
                                                 bias=bias_comb[:, ct, b:b + 1])

            # ---------- phase B: conv -> x (masked) + stats ----------
            x = bigp.tile([P, 2, B, SP], BF16)
            sumx = smp.tile([P, 2, 12], F32, tag="sumx")
            sumsq = smp.tile([P, 2, 12], F32, tag="sumsq")
            NCH = SP // 3  # 396 = 6 rows
            taps = [(dy, dx) for dy in range(3) for dx in range(3)]
            for rc in range(3):
                for b in range(B):
                    for co in range(2):
                        pg = psp.tile([P, NCH], F32, tag="mm")
                        for ti, (dy, dx) in enumerate(taps):
                            sh = (dy - 1) * SC + (dx - 1)
                            off = GUARD + b * SP + rc * NCH + sh
                            for ci in range(2):
                                nc.tensor.matmul(
                                    pg[:],
                                    convw[:, ti, ci, co * P:(co + 1) * P],
                                    et[:, ci, off:off + NCH],
                                    start=(ti == 0 and ci == 0),
                                    stop=(ti == 8 and ci == 1))
                        idx = b * 3 + rc
                        nc.vector.scalar_tensor_tensor(
                            out=x[:, co, b, rc * NCH:(rc + 1) * NCH],
                            in0=pg[:], scalar=convb[:, co:co + 1],
                            in1=mask[:, b * SP + rc * NCH:b * SP + (rc + 1) * NCH],
                            op0=ALU.add, op1=ALU.mult,
                            accum_out=sumx[:, co, idx:idx + 1])
                        sq = sqp.tile([P, NCH], BF16, tag="sq")
                        nc.scalar.activation(sq[:], x[:, co, b, rc * NCH:(rc + 1) * NCH],
                                             AF.Square, accum_out=sumsq[:, co, idx:idx + 1])

            # ---------- BN stats AllReduce ----------
            stats = smp.tile([P, 4], F32, tag="stats")
            nc.vector.tensor_reduce(stats[:, 0:1], sumx[:, 0, :], axis=AX.X, op=ALU.add)
            nc.vector.tensor_reduce(stats[:, 1:2], sumx[:, 1, :], axis=AX.X, op=ALU.add)
            nc.vector.tensor_reduce(stats[:, 2:3], sumsq[:, 0, :], axis=AX.X, op=ALU.add)
            nc.vector.tensor_reduce(stats[:, 3:4], sumsq[:, 1, :], axis=AX.X, op=ALU.add)
            st_in = dramp.tile([P, 4], F32)
            st_out = dramp.tile([P, 4], F32)
            nc.gpsimd.dma_start(st_in[:], stats[:])
            nc.gpsimd.collective_compute(
                "AllReduce", ALU.add,
                replica_groups=[list(range(N_CORES))],
                ins=[st_in.opt()], outs=[st_out.opt()])
            gstats = smp.tile([P, 4], F32, tag="gstats")
            nc.gpsimd.dma_start(gstats[:], st_out[:])

            bng = load(wp, bng_e)
            bnb = load(wp, bnb_e)
            mu = smp.tile([P, 2], F32, tag="mu")
            nc.scalar.activation(mu[:], gstats[:, 0:2], AF.Copy, scale=1.0 / NTOT)
            ex2 = smp.tile([P, 2], F32, tag="ex2")
            nc.scalar.activation(ex2[:], gstats[:, 2:4], AF.Copy, scale=1.0 / NTOT)
            musq = smp.tile([P, 2], F32, tag="musq")
            nc.scalar.activation(musq[:], mu[:], AF.Square)
            var = smp.tile([P, 2], F32, tag="var")
            nc.vector.tensor_tensor(out=var[:], in0=ex2[:], in1=musq[:], op=ALU.subtract)
            veps = smp.tile([P, 2], F32, tag="veps")
            nc.vector.tensor_scalar_add(veps[:], var[:], 1e-5)
            stdv = smp.tile([P, 2], F32, tag="stdv")
            nc.scalar.activation(stdv[:], veps[:], AF.Sqrt)
            inv = smp.tile([P, 2], F32, tag="inv")
            nc.vector.reciprocal(inv[:], stdv[:])
            svec = smp.tile([P, 2], F32, tag="svec")
            nc.vector.tensor_tensor(out=svec[:], in0=inv[:], in1=bng[:], op=ALU.mult)
            msv = smp.tile([P, 2], F32, tag="msv")
            nc.vector.tensor_tensor(out=msv[:], in0=mu[:], in1=svec[:], op=ALU.mult)
            tvec = smp.tile([P, 2], F32, tag="tvec")
            nc.vector.tensor_tensor(out=tvec[:], in0=bnb[:], in1=msv[:], op=ALU.subtract)

            # ---------- BN + tanh (in place on x) ----------
            for co in range(2):
                nc.scalar.activation(x[:, co, :, :], x[:, co, :, :], AF.Tanh,
                                     bias=tvec[:, co:co + 1], scale=svec[:, co:co + 1])

            # ---------- scores -> exp -> masked softmax ----------
            vattw = load(wp, vattw_e)
            vattb = load(wp, vattb_e)
            e_row = bigp.tile([1, B * SP], F32)
            for b in range(B):
                for sc3 in range(3):
                    pss = psp.tile([1, NCH], F32, tag="mm")
                    for co in range(2):
                        nc.tensor.matmul(pss[:], vattw[:, co:co + 1],
                                         x[:, co, b, sc3 * NCH:(sc3 + 1) * NCH],
                                         start=(co == 0), stop=(co == 1))
                    nc.scalar.activation(e_row[0:1, b * SP + sc3 * NCH:b * SP + (sc3 + 1) * NCH],
                                         pss[:], AF.Exp, bias=vattb[0:1, 0:1])
            denom = smp.tile([1, B], F32, tag="denom")
            for b in range(B):
                nc.vector.scalar_tensor_tensor(
                    out=e_row[0:1, b * SP:(b + 1) * SP],
                    in0=e_row[0:1, b * SP:(b + 1) * SP], scalar=0.0,
                    in1=mask[0:1, b * SP:(b + 1) * SP],
                    op0=ALU.add, op1=ALU.mult,
                    accum_out=denom[0:1, b:b + 1])
            recip = smp.tile([1, B], F32, tag="recip")
            nc.vector.reciprocal(recip[:], denom[:])
            for b in range(B):
                nc.scalar.activation(e_row[0:1, b * SP:(b + 1) * SP],
                                     e_row[0:1, b * SP:(b + 1) * SP],
                                     AF.Copy, scale=recip[0:1, b:b + 1])

            # alpha DMA out (interior)
            for b in range(B):
                al_in = e_row[0:1, b * SP:(b + 1) * SP].rearrange(
                    "o (r c) -> o r c", c=SC)[:, 1:1 + HD, 1:1 + WD]
                nc.gpsimd.dma_start(al_e[b:b + 1, :], al_in)

            # ---------- alpha -> bf16 interior row, then broadcast ----------
            alpha_bf = wp.tile([1, B * HW], BF16, tag="ld_att_bf")  # reuse att slot
            for b in range(B):
                a_in = e_row[0:1, b * SP:(b + 1) * SP].rearrange(
                    "o (r c) -> o r c", c=SC)[:, 1:1 + HD, 1:1 + WD]
                nc.scalar.activation(
                    alpha_bf[0:1, b * HW:(b + 1) * HW].rearrange(
                        "o (r c) -> o r c", c=WD),
                    a_in, AF.Copy)
            alpha_bc = bigp.tile([P, B, HW], BF16, tag="maskslot")
            for b in range(B):
                for half in range(2):
                    pb = psp.tile([P, 512], F32, tag="mm")
                    nc.tensor.matmul(pb[:], ones[0:1, :],
                                     alpha_bf[0:1, b * HW + half * 512:b * HW + (half + 1) * 512],
                                     start=True, stop=True)
                    nc.scalar.activation(alpha_bc[:, b, half * 512:(half + 1) * 512],
                                         pb[:], AF.Copy)

            # ---------- ct via DVE multiply-reduce ----------
            ctT = smp.tile([P, 8, B], F32, tag="ctT")
            for b in range(B):
                for half in range(2):
                    enc2 = enc2p.tile([P, 4, HW], BF16, tag="enc2")
                    nc.gpsimd.dma_start(
                        enc2[:],
                        enc_e[b].rearrange("(t p) hw -> p t hw", p=P)[:, half * 4:(half + 1) * 4, :])
                    for ci in range(4):
                        ct = half * 4 + ci
                        scr = sqp.tile([P, HW], BF16, tag="ttr_scr")
                        nc.vector.scalar_tensor_tensor(
                            out=scr[:], in0=enc2[:, ci, :], scalar=1.0,
                            in1=alpha_bc[:, b, :],
                            op0=ALU.mult, op1=ALU.mult,
                            accum_out=ctT[:, ct, b:b + 1])
            ct_bf = smp.tile([P, 8, B], BF16, tag="ct_bf")
            nc.scalar.activation(ct_bf[:], ctT[:], AF.Copy)

            # ---------- GRU2 ----------
            gwi = wp.tile([P, 8, 768], BF16, tag="g1wi")
            nc.gpsimd.dma_start(gwi[:], gwi_e[...])
            gwh = wp.tile([P, 2, 768], BF16, tag="g1wh")
            nc.gpsimd.dma_start(gwh[:], gwh_e[...])
            gwv = wp.tile([P, 6, 768], BF16, tag="g1wv")
            nc.gpsimd.dma_start(gwv[:], gwv_e[...])
            v1_row = smp.tile([B, 768], F32, tag="v1row")
            h1_row = smp.tile([B, 256], F32, tag="h1row")
            gru_cell(8, ct_bf, gwi, stT_bf, gwh, 6, vtT_bf, gwv, v1_row, st_row, h1_row)
            nc.gpsimd.dma_start(h1_e[...], h1_row[:])
            nc.gpsimd.dma_start(v1_e[...], v1_row[:])

            # ---------- logits ----------
            h2w = load(wp, h2w_e)
            e2w = load(wp, e2w_e)
            wcw = load(wp, wcw_e)
            preb = load(wp, preb_e)
            outw = load(wp, outw_e)
            outb_bf = load(wp, outb_bf_e)
            h1T_bf = transpose_to_bf(h1_row, 2, "h1T")
            pp = psp.tile([P, B], F32, tag="mm")
            first = True
            for kt in range(2):
                nc.tensor.matmul(pp[:], h2w[:, kt, :], h1T_bf[:, kt, :], start=first, stop=False)
                first = False
            for kt in range(2):
                nc.tensor.matmul(pp[:], e2w[:, kt, :], embT_bf[:, kt, :], start=False, stop=False)
            for kt in range(8):
                nc.tensor.matmul(pp[:], wcw[:, kt, :], ct_bf[:, kt, :],
                                 start=False, stop=(kt == 7))
            preT_bf = smp.tile([P, B], BF16, tag="preT")
            nc.scalar.activation(preT_bf[:], pp[:], AF.Identity, bias=preb[:, 0:1])

            pl = psp.tile([B, VOCAB], F32, tag="mm")
            nc.tensor.matmul(pl[:], preT_bf[:], outw[:], start=True, stop=False)
            nc.tensor.matmul(pl[:], ones4_bf[0:1, :], outb_bf[0:1, :], start=False, stop=True)

            # log_softmax over free dim
            mx = smp.tile([B, 1], F32, tag="mx")
            nc.vector.tensor_reduce(mx[:], pl[:], axis=AX.X, op=ALU.max)
            nmx = smp.tile([B, 1], F32, tag="nmx")
            nc.vector.tensor_scalar_mul(nmx[:], mx[:], -1.0)
            es = smp.tile([B, VOCAB], F32, tag="es")
            ssum = smp.tile([B, 1], F32, tag="ssum")
            nc.scalar.activation(es[:], pl[:], AF.Exp, bias=nmx[:, 0:1], accum_out=ssum[:, 0:1])
            lse = smp.tile([B, 1], F32, tag="lse")
            nc.scalar.activation(lse[:], ssum[:], AF.Ln)
            t1 = smp.tile([B, VOCAB], F32, tag="t1")
            nc.vector.tensor_scalar_sub(t1[:], pl[:], mx[:, 0:1])
            o_row = smp.tile([B, VOCAB], F32, tag="orow")
            nc.vector.tensor_scalar_sub(o_row[:], t1[:], lse[:, 0:1])
            nc.gpsimd.dma_start(o_e[...], o_row[:])

    nc.compile()
    return nc


def _get_nc():
    global _CACHED
    if _CACHED is None:
        _CACHED = _build()
    return _CACHED


def _conv1_host(da, w, b):
    # 3x3 conv, 1->1 channel, SAME, NCHW; da: (32,1,16,64)
    x = da[:, 0]
    xp = np.pad(x, ((0, 0), (1, 1), (1, 1)))
    out = np.zeros_like(x)
    for dy in range(3):
        for dx in range(3):
            out += w[0, 0, dy, dx] * xp[:, dy:dy + 16, dx:dx + 64]
    return (out + b[0])[:, None].astype(np.float32)


def _prep(params, input_a, hidden, v, encoder_outputs, attention_sum,
          decoder_attention, h_mask, w_mask):
    def ga(x):
        return np.asarray(x, dtype=np.float32)

    BT = 32
    input_a = np.asarray(input_a)
    h_mask = np.asarray(h_mask)
    w_mask = np.asarray(w_mask)
    enc_full = ga(encoder_outputs).reshape(BT, ENC, HW)
    att_sum_out = ga(attention_sum) + _conv1_host(
        ga(decoder_attention), ga(params['conv1']['w']), ga(params['conv1']['b']))
    emb_full = ga(params['embedding'])[input_a]          # (32, 256)
    hid_full = ga(hidden).reshape(BT, HSZ)
    v_full = ga(v).reshape(BT, 3 * HSZ)

    # masks in padded layout
    m = ((np.arange(HD)[None, :, None] < h_mask[:, None, None]) &
         (np.arange(WD)[None, None, :] < w_mask[:, None, None])).astype(np.float32)
    mpad = np.zeros((BT, SR, SC), np.float32)
    mpad[:, 1:1 + HD, 1:1 + WD] = m

    bf = ml_dtypes.bfloat16

    def t_bf(a2d, ncols):  # (K, M) f32 -> [128, ncols, M] for K=ncols*128 lhsT tiles? no:
        # a2d: (ncols*128, M) -> [128, ncols, M]
        K, M = a2d.shape
        return np.ascontiguousarray(a2d.reshape(ncols, P, M).transpose(1, 0, 2)).astype(bf)

    def t_f32(a2d, ncols):
        K, M = a2d.shape
        return np.ascontiguousarray(a2d.reshape(ncols, P, M).transpose(1, 0, 2)).astype(np.float32)

    def vec2(a):  # (256,) -> [128, 2]
        return np.ascontiguousarray(a.reshape(2, P).T).astype(np.float32)

    g1 = params['gru1']
    g2 = params['gru']
    convw_np = ga(params['conv_tan']['w'])  # (co, ci, 3, 3)
    convw_t = np.transpose(convw_np, (2, 3, 1, 0)).reshape(9, 256, 256)  # (tap, ci, co)
    convw_up = np.ascontiguousarray(
        convw_t.reshape(9, 2, P, 256).transpose(2, 0, 1, 3)).astype(bf)  # [128, 9, 2, 256]

    shared = {
        "g1wi": t_bf(ga(g1['wi']), 2),
        "g1wh": t_bf(ga(g1['wh']), 2),
        "g1wv": t_bf(ga(g1['wv']), 6),
        "gwi": t_bf(ga(g2['wi']), 8),
        "gwh": t_bf(ga(g2['wh']), 2),
        "gwv": t_bf(ga(g2['wv']), 6),
        "hidw": t_bf(ga(params['hidden']['w']), 2),
        "hidb": vec2(ga(params['hidden']['b'])),
        "uaw": t_bf(ga(params['ua']['w']), 8),
        "uaufb": vec2(ga(params['ua']['b']) + ga(params['uf']['b'])),
        "ufw": ga(params['uf']['w']).reshape(1, 256).astype(bf),
        "convw": convw_up,
        "convb": vec2(ga(params['conv_tan']['b'])),
        "bng": vec2(ga(params['bn1_gamma'])),
        "bnb": vec2(ga(params['bn1_beta'])),
        "vattw": np.ascontiguousarray(
            ga(params['v_att']['w']).reshape(2, P).T).astype(bf),
        "vattb": ga(params['v_att']['b']).reshape(1, 1),
        "h2w": t_bf(ga(params['hidden2']['w']), 2),
        "e2w": t_bf(ga(params['emb2']['w']), 2),
        "wcw": t_bf(ga(params['wc']['w']), 8),
        "preb": (ga(params['hidden2']['b']) + ga(params['emb2']['b'])
                 + ga(params['wc']['b'])).reshape(P, 1).astype(np.float32),
        "outw": ga(params['out']['w']).astype(bf),
        "outb_bf": ga(params['out']['b']).reshape(1, VOCAB).astype(bf),
        "ones": np.ones((1, P), bf),
        "ones4_bf": np.ones((1, B), bf),
        "id4": np.eye(B, dtype=np.float32),
    }

    in_maps = []
    for c in range(N_CORES):
        sl = slice(c * B, (c + 1) * B)
        im = dict(shared)
        im["enc"] = enc_full[sl].astype(bf)
        im["embT_bf"] = t_bf(emb_full[sl].T, 2)
        im["hT_bf"] = t_bf(hid_full[sl].T, 2)
        im["vT_bf"] = t_bf(v_full[sl].T, 6)
        im["h_row"] = np.ascontiguousarray(hid_full[sl])
        im["att_bf"] = att_sum_out[sl, 0].reshape(1, B * HW).astype(bf)
        im["mask"] = np.broadcast_to(
            mpad[sl].reshape(1, B * SP), (P, B * SP)).copy()
        in_maps.append(im)
    return in_maps, att_sum_out


def kernel(params, input_a, hidden, v, encoder_outputs, attention_sum,
           decoder_attention, h_mask, w_mask, bb, dense_input,
           batch_size, gpu, epoch):
    in_maps, att_sum_out = _prep(params, input_a, hidden, v, encoder_outputs,
                                 attention_sum, decoder_attention, h_mask, w_mask)
    nc = _get_nc()
    trace = bool(int(os.environ.get("KERNEL_TRACE", "0")))
    res = run_bass_kernel_spmd(nc, in_maps, core_ids=list(range(N_CORES)),
                               trace=trace)
    global LAST_EXEC_NS
    LAST_EXEC_NS = res.exec_time_ns

    out = np.concatenate([r["o_out"][:, None, :] for r in res.results], 0)
    h1 = np.concatenate([r["h1_out"][:, None, :] for r in res.results], 0)
    v1 = np.concatenate([r["v1_out"][:, None, :] for r in res.results], 0)
    alpha = np.concatenate(
        [r["alpha_out"].reshape(B, 1, HD, WD) for r in res.results], 0)
    return (out.astype(np.float32), h1.astype(np.float32), v1.astype(np.float32),
            alpha.astype(np.float32), att_sum_out.astype(np.float32))
